# revision 35
# baseline (speedup 1.0000x reference)
"""Grouped MoE (top-2 of 8 experts, SwiGLU) on 8 Trainium2 NeuronCores.

Sharding: expert-parallel with real token dispatch. The gate (softmax +
top-2 + renormalize) is computed on host as part of the sharding step;
tokens are gathered per expert into fixed-capacity buffers (CAP = max
expert count rounded up to 64). Core c owns expert c and runs the three
SwiGLU GEMMs in bf16 over only its own ~T*K/E tokens, scales each output
row by that token's gate weight on-device, and writes its [CAP, D] bf16
shard. The host scatter-adds the two expert contributions per token back
into the full [T, D] output. No collectives are needed: each token's two
expert rows live on different cores and are summed on host.
"""

import sys
import numpy as np

for _p in ("/opt/trn_rl_repo",):
    if _p not in sys.path:
        sys.path.insert(0, _p)

B, S, D, F, E, K = 2, 2048, 1024, 1024, 8, 2
T = B * S            # 4096 tokens
NCORES = 8
P = 128
DK = D // P          # 8 contraction chunks over D
FK = F // P          # 8 F tiles
MAXCH = 512          # max token chunk (PSUM bank limit: 512 f32/partition)

_cache = {}


def _build_nc(cap):
    from contextlib import ExitStack

    import concourse.mybir as mybir
    import concourse.tile as tile
    from concourse import bacc

    dt = mybir.dt
    AF = mybir.ActivationFunctionType
    ALU = mybir.AluOpType

    ntiles = (cap + P - 1) // P
    # Token chunks of up to 512 f32 (PSUM bank limit).  Interior chunk
    # boundaries stay on the 128 grid (B-stage tiles index the global gate /
    # output layout); any sub-128 remainder rides at the end of the final
    # chunk, kept >= 128 wide so no matmul drops near the ~60-cycle NX
    # dispatch floor.
    tail = cap % P
    base = cap - tail
    f128 = min(base // P, (MAXCH - tail) // P) if tail else 0
    # keep every chunk >= 256 when possible: below that the ~40ns LDWEIGHTS
    # is no longer hidden behind the matmul stream (n=128 pairs run ~70-81ns
    # vs the 56ns streaming ideal)
    while f128 > 1:
        r = (base - f128 * P) % MAXCH
        if r == 0 or r >= 2 * P:
            break
        f128 -= 1
    rest = base - f128 * P
    sizes = [MAXCH] * (rest // MAXCH)
    if rest % MAXCH:
        sizes.append(rest % MAXCH)
    if tail or f128:
        sizes.append(f128 * P + tail)
    chunks = []
    off = 0
    for tch in sizes:
        chunks.append((off, tch))
        off += tch
    assert sum(sizes) == cap and all(0 < s <= MAXCH for s in sizes)

    nc = bacc.Bacc("TRN2", target_bir_lowering=False, debug=False,
                   num_devices=NCORES)

    xg = nc.dram_tensor("xg", [D, cap], dt.bfloat16, kind="ExternalInput").ap()
    gsc = nc.dram_tensor("gsc", [P, ntiles], dt.float32,
                         kind="ExternalInput").ap()
    # w1/w3 arrive host-packed as [k-tile][column-half] contiguous 128 KB
    # blocks (4 KB DMA packets instead of 1 KB lines -> full HBM rate)
    w1t = nc.dram_tensor("w1t", [DK * 2 * P, F // 2], dt.bfloat16,
                         kind="ExternalInput").ap()
    w3t = nc.dram_tensor("w3t", [DK * 2 * P, F // 2], dt.bfloat16,
                         kind="ExternalInput").ap()
    w2t = nc.dram_tensor("w2t", [F, D], dt.bfloat16, kind="ExternalInput").ap()
    out = nc.dram_tensor("out", [cap, D], dt.bfloat16,
                         kind="ExternalOutput").ap()

    with tile.TileContext(nc) as tc, ExitStack() as ctx:
        const = ctx.enter_context(tc.tile_pool(name="const", bufs=1))
        xpool = ctx.enter_context(tc.tile_pool(name="xpool", bufs=1))
        spool = ctx.enter_context(tc.tile_pool(name="spool", bufs=2))
        hpool = ctx.enter_context(tc.tile_pool(name="hpool", bufs=2))
        ypool = ctx.enter_context(tc.tile_pool(name="ypool", bufs=3))

        abpsum = ctx.enter_context(tc.tile_pool(name="abpsum", bufs=2,
                                                space="PSUM"))
        ypsum = ctx.enter_context(tc.tile_pool(name="ypsum", bufs=2,
                                               space="PSUM"))

        # ---- resident loads, all on the sync HWDGE ring in criticality
        # order (x chunk 0 as one multi-AP DMA, then w1/w3 low halves, the
        # high halves, gate scales, w2 and the x tail).  The scalar engine
        # stays DMA-free so silu/copy are never queued behind descriptor
        # issues. ----
        xall = xpool.tile([P, DK * cap], dt.bfloat16, tag="xall")
        xg_sb = [xall[:, k * cap:(k + 1) * cap] for k in range(DK)]
        t0, tch0 = chunks[0]
        nc.sync.dma_start(
            xall[:].rearrange("p (k t) -> p k t", k=DK)[:, :, t0:tch0],
            xg.rearrange("(k p) t -> p k t", p=P)[:, :, t0:tch0])

        # w1/w3 arrive in column halves: the low halves (2 MB, serving F-tiles
        # 0..3) are the only weight bytes on the startup critical path; the
        # high halves stream in while f=0..3 compute.
        w1_sb = [const.tile([P, F], dt.bfloat16, tag=f"w1_{k}",
                            name=f"w1_{k}") for k in range(DK)]
        w3_sb = [const.tile([P, F], dt.bfloat16, tag=f"w3_{k}",
                            name=f"w3_{k}") for k in range(DK)]
        for k in range(DK):
            nc.sync.dma_start(w1_sb[k][:, 0:F // 2],
                              w1t[2 * k * P:(2 * k + 1) * P, :])
        for k in range(DK):
            nc.sync.dma_start(w3_sb[k][:, 0:F // 2],
                              w3t[2 * k * P:(2 * k + 1) * P, :])
        for k in range(DK):
            nc.sync.dma_start(w1_sb[k][:, F // 2:F],
                              w1t[(2 * k + 1) * P:(2 * k + 2) * P, :])
        for k in range(DK):
            nc.sync.dma_start(w3_sb[k][:, F // 2:F],
                              w3t[(2 * k + 1) * P:(2 * k + 2) * P, :])

        gsc_sb = const.tile([P, ntiles], dt.float32, tag="gsc")
        nc.sync.dma_start(gsc_sb[:], gsc[:, :])

        # w2 in two half-transfers so stage B's first accumulation (fk=0..3)
        # does not wait for the full 2 MB
        w2all = const.tile([P, FK * D], dt.bfloat16, tag="w2all")
        for g in range(2):
            nc.sync.dma_start(
                w2all[:].rearrange("p (k d) -> p k d",
                                   k=FK)[:, g * 4:(g + 1) * 4, :],
                w2t.rearrange("(k p) d -> p k d", p=P)[:, g * 4:(g + 1) * 4, :])
        w2_sb = [w2all[:, k * D:(k + 1) * D] for k in range(FK)]

        if cap > tch0:
            nc.sync.dma_start(
                xall[:].rearrange("p (k t) -> p k t", k=DK)[:, :, tch0:cap],
                xg.rearrange("(k p) t -> p k t", p=P)[:, :, tch0:cap])

        # ---- PE warm-up: dummy matmuls while the weight DMAs are in flight
        # keep the tensor engine's activity window full so HAM reaches the
        # 2.4 GHz p-state before the real stream begins ----
        wrm = spool.tile([P, 512], dt.bfloat16, tag="wrm")
        nc.vector.memset(wrm[:], 0.5)
        psW = abpsum.tile([P, 512], dt.float32, tag="psA")
        for _ in range(18):
            nc.tensor.matmul(psW[:], lhsT=wrm[:, 0:P], rhs=wrm[:],
                             start=True, stop=True)

        # ---- per-chunk SwiGLU FFN ----
        for (tok, tch) in chunks:
            h_sb = []
            for f in range(FK):
                psA = abpsum.tile([P, tch], dt.float32, tag="psA")
                for k in range(DK):
                    nc.tensor.matmul(
                        psA[:], lhsT=w1_sb[k][:, f * P:(f + 1) * P],
                        rhs=xg_sb[k][:, tok:tok + tch],
                        start=(k == 0), stop=(k == DK - 1))
                psB = abpsum.tile([P, tch], dt.float32, tag="psB")
                for k in range(DK):
                    nc.tensor.matmul(
                        psB[:], lhsT=w3_sb[k][:, f * P:(f + 1) * P],
                        rhs=xg_sb[k][:, tok:tok + tch],
                        start=(k == 0), stop=(k == DK - 1))
                ssb = spool.tile([P, tch], dt.bfloat16, tag="ssb")
                nc.scalar.activation(ssb[:], psA[:], AF.Silu)
                hsb = hpool.tile([P, tch], dt.bfloat16, tag=f"h{f}")
                nc.vector.tensor_tensor(hsb[:], ssb[:], psB[:], op=ALU.mult)
                h_sb.append(hsb)
            nm = (tch + P - 1) // P
            morder = list(range(nm))
            if tch % P:
                # emit the partial tile first so the kernel's final
                # copy+store chain hangs off a full tile with no over-wait
                morder = [nm - 1] + morder[:-1]
            for m in morder:
                jj = tok // P + m
                pm = min(P, tch - m * P)
                psY = ypsum.tile([P, D], dt.float32, tag="psY")
                for nhalf in range(2):
                    for fk in range(FK):
                        nc.tensor.matmul(
                            psY[:pm, nhalf * 512:(nhalf + 1) * 512],
                            lhsT=h_sb[fk][:, m * P:m * P + pm],
                            rhs=w2_sb[fk][:, nhalf * 512:(nhalf + 1) * 512],
                            start=(fk == 0), stop=(fk == FK - 1))
                ysb = ypool.tile([P, D], dt.bfloat16, tag="ysb")
                nc.scalar.activation(ysb[:pm, :], psY[:pm, :], AF.Copy,
                                     scale=gsc_sb[:pm, jj:jj + 1])
                nc.sync.dma_start(out[jj * P:jj * P + pm, :], ysb[:pm, :])

    nc.compile()
    return nc


def _route(xf, gate_w):
    """Host gate: returns per-expert (token indices, renormalized weights)."""
    logits = xf.astype(np.float64) @ gate_w.astype(np.float64).T   # [T, E]
    order = np.argsort(-logits, axis=1, kind="stable")
    i1 = order[:, 0]
    i2 = order[:, 1]
    ar = np.arange(T)
    l1 = logits[ar, i1]
    l2 = logits[ar, i2]
    g1 = 1.0 / (1.0 + np.exp(l2 - l1))
    g2 = 1.0 - g1
    idx_e, scl_e = [], []
    for e in range(E):
        m1 = i1 == e
        m2 = i2 == e
        ids = np.concatenate([np.nonzero(m1)[0], np.nonzero(m2)[0]])
        sc = np.concatenate([g1[m1], g2[m2]])
        idx_e.append(ids)
        scl_e.append(sc.astype(np.float32))
    return idx_e, scl_e


def prepare(x, gate_w, w1, w3, w2):
    """Host routing + sharding: returns (nc, in_maps, idx_e)."""
    import ml_dtypes

    xf = np.ascontiguousarray(x.reshape(T, D).astype(np.float32))
    xTb = np.ascontiguousarray(xf.T).astype(ml_dtypes.bfloat16)   # [D, T]

    idx_e, scl_e = _route(xf, gate_w)
    maxcnt = max(len(i) for i in idx_e)
    cap = ((maxcnt + 3) // 4) * 4     # 4-token grain keeps DMA rows 8B-aligned
    ntiles = (cap + P - 1) // P

    if cap not in _cache:
        _cache[cap] = _build_nc(cap)
    nc = _cache[cap]

    in_maps = []
    for c in range(NCORES):
        ids = idx_e[c]
        cnt = len(ids)
        xg_c = np.zeros((D, cap), dtype=ml_dtypes.bfloat16)
        xg_c[:, :cnt] = xTb[:, ids]
        sc = np.zeros(ntiles * P, dtype=np.float32)
        sc[:cnt] = scl_e[c]
        gsc_c = np.ascontiguousarray(sc.reshape(ntiles, P).T)     # [P, ntiles]
        def _pack(wT):
            # [D, F] -> [(k, half, p), F/2] so each [128, 512] half-tile DMA
            # reads one contiguous 128 KB block
            return np.ascontiguousarray(
                wT.reshape(DK, P, 2, F // 2).transpose(0, 2, 1, 3)
                .reshape(DK * 2 * P, F // 2))

        in_maps.append({
            "xg": xg_c,
            "gsc": gsc_c,
            "w1t": _pack(np.ascontiguousarray(w1[c].T)
                         .astype(ml_dtypes.bfloat16)),
            "w3t": _pack(np.ascontiguousarray(w3[c].T)
                         .astype(ml_dtypes.bfloat16)),
            "w2t": np.ascontiguousarray(w2[c].T).astype(ml_dtypes.bfloat16),
        })
    return nc, in_maps, idx_e


def _combine(res, idx_e):
    outf = np.zeros((T, D), dtype=np.float32)
    for c in range(NCORES):
        cnt = len(idx_e[c])
        outf[idx_e[c]] += res.results[c]["out"][:cnt].astype(np.float32)
    return outf.reshape(B, S, D)


def kernel(x, gate_w, w1, w3, w2):
    from concourse.bass_utils import run_bass_kernel_spmd

    nc, in_maps, idx_e = prepare(x, gate_w, w1, w3, w2)
    res = run_bass_kernel_spmd(nc, in_maps, list(range(NCORES)))
    return _combine(res, idx_e)
